# revision 1
# baseline (speedup 1.0000x reference)
"""Trainium2 Bass kernel for nn_JaCDEManual_13829794693220.

Computes h_dot for the RNN-cell Jacobian Neumann series:
    x    = cubic_spline(coeffs, tobs, t)           [B, C]
    xdot = cubic_spline(dcoeffs, tobs, t)          [B, C]
    l1   = x @ wx.T + h @ wh.T + b0                [B, H]
    tanh = tanh(relu(l1) @ wout.T + b1)
    d_outer = diag(1-tanh^2) wout diag(sigmoid(l1))   (per batch row)
    h_dot = sum_{k=0..8} (d_outer wh)^k (d_outer wx xdot)

Key algebra: d_outer @ v = dtanh * (wout @ (drelu * v)), so no [B,H,H]
tensor is ever materialized; everything is [128,128] @ [128,512] matmuls
plus elementwise scalings.

Sharding: pure data parallel over batch B=4096 -> 8 cores x 512 rows.
Activations live transposed on chip ([H=128 partitions, batch free]); the
host pre-transposes the per-core input slices / weights (layout only) and
folds the degree-4 spline combination matrix A(dt) into P = A @ wx.T so
the spline eval + wx projection is a single matmul per tensor.
"""

import os
import sys

import numpy as np

for _p in (
    "/root/.axon_site",
    "/root/.axon_site/_ro/trn_rl_repo",
    "/root/.axon_site/_ro/pypackages",
    "/opt/trn_rl_repo",
):
    if os.path.isdir(_p) and _p not in sys.path:
        sys.path.append(_p)

import concourse.bacc as bacc
import concourse.mybir as mybir
import concourse.tile as tile
from concourse import bass_utils

B, H, C = 4096, 128, 32
N_CORES = 8
BL = B // N_CORES  # 512 batch rows per core
HALF = BL // 2
K_TERMS = 8
F32 = mybir.dt.float32
AF = mybir.ActivationFunctionType

# PE matmul operand dtype: fp32 is exact but lowered as 2 half-speed passes
# (4 cyc/row); float32r streams 1 cyc/row for free dim >= 256.
_MM_DT = {
    "fp32": mybir.dt.float32,
    "fp32r": mybir.dt.float32r,
}[os.environ.get("KERNEL_MM_DTYPE", "fp32")]


def _mm(nc, out, lhsT, rhs, **kw):
    nc.tensor.matmul(out, lhsT.bitcast(_MM_DT), rhs.bitcast(_MM_DT), **kw)


def _body(tc, out, hT, kcT, dkcT, P, whT, woutT, b0, b1):
    from contextlib import ExitStack

    nc = tc.nc
    with ExitStack() as ctx:
        const = ctx.enter_context(tc.tile_pool(name="const", bufs=1))
        data = ctx.enter_context(tc.tile_pool(name="data", bufs=1))
        acts = ctx.enter_context(tc.tile_pool(name="acts", bufs=1))
        loop_sb = ctx.enter_context(tc.tile_pool(name="loop_sb", bufs=2))
        ps_pre = ctx.enter_context(tc.tile_pool(name="ps_pre", bufs=1, space="PSUM"))
        ps_loop = ctx.enter_context(tc.tile_pool(name="ps_loop", bufs=1, space="PSUM"))

        P_sb = const.tile([4 * C, H], F32)
        nc.sync.dma_start(out=P_sb, in_=P)
        whT_sb = const.tile([H, H], F32)
        nc.sync.dma_start(out=whT_sb, in_=whT)
        woutT_sb = const.tile([H, H], F32)
        nc.sync.dma_start(out=woutT_sb, in_=woutT)
        b0_sb = const.tile([H, 1], F32)
        nc.sync.dma_start(out=b0_sb, in_=b0)
        b1_sb = const.tile([H, 1], F32)
        nc.sync.dma_start(out=b1_sb, in_=b1)

        hT_sb = data.tile([H, BL], F32)
        nc.sync.dma_start(out=hT_sb, in_=hT)
        kcT_sb = data.tile([4 * C, BL], F32)
        nc.sync.dma_start(out=kcT_sb, in_=kcT)
        dkcT_sb = data.tile([4 * C, BL], F32)
        nc.sync.dma_start(out=dkcT_sb, in_=dkcT)

        # l1.T = (wx @ A.T) @ kcT + wh @ hT   (+ b0 via activation bias)
        l1 = ps_pre.tile([H, BL], F32, tag="l1")
        _mm(nc, l1, P_sb, kcT_sb, start=True, stop=False)
        _mm(nc, l1, whT_sb, hT_sb, start=False, stop=True)

        relu = acts.tile([H, BL], F32)
        nc.scalar.activation(relu, l1, AF.Relu, bias=b0_sb)
        drelu = acts.tile([H, BL], F32)
        nc.scalar.activation(drelu, l1, AF.Sigmoid, bias=b0_sb)

        a2 = ps_pre.tile([H, BL], F32, tag="a2")
        _mm(nc, a2, woutT_sb, relu, start=True, stop=True)
        tanh_sb = acts.tile([H, BL], F32)
        nc.scalar.activation(tanh_sb, a2, AF.Tanh, bias=b1_sb)
        dtanh = acts.tile([H, BL], F32)
        nc.vector.tensor_mul(dtanh, tanh_sb, tanh_sb)
        nc.vector.tensor_scalar(
            out=dtanh,
            in0=dtanh,
            scalar1=-1.0,
            scalar2=1.0,
            op0=mybir.AluOpType.mult,
            op1=mybir.AluOpType.add,
        )

        # u.T = (wx @ A.T) @ dkcT ; g0 = drelu * u
        u = ps_pre.tile([H, BL], F32, tag="u")
        _mm(nc, u, P_sb, dkcT_sb, start=True, stop=True)

        g = []
        for hh in range(2):
            sl = slice(hh * HALF, (hh + 1) * HALF)
            gt = loop_sb.tile([H, HALF], F32, tag=f"g{hh}", name=f"g{hh}_init")
            nc.vector.tensor_mul(gt, drelu[:, sl], u[:, sl])
            g.append(gt)

        # Neumann loop. S accumulates sum_k wout @ g_k in PSUM via
        # duplicate matmuls; h_dot = dtanh * S at the end.
        S = ps_loop.tile([H, BL], F32, tag="S")
        for k in range(K_TERMS + 1):
            last = k == K_TERMS
            y = None
            if not last:
                y = ps_loop.tile([H, BL], F32, tag="y", name=f"y_{k}")
            for hh in range(2):
                sl = slice(hh * HALF, (hh + 1) * HALF)
                if not last:
                    _mm(nc, y[:, sl], woutT_sb, g[hh], start=True, stop=True)
                # start only once: start=True marks the whole 2KB PSUM zero
                # region pending-zero, so a second start on this bank would
                # wipe the other half's partial sum.
                _mm(
                    nc,
                    S[:, sl],
                    woutT_sb,
                    g[hh],
                    start=(k == 0 and hh == 0),
                    stop=(last and hh == 1),
                )
            if last:
                break
            m = []
            for hh in range(2):
                sl = slice(hh * HALF, (hh + 1) * HALF)
                mt = loop_sb.tile([H, HALF], F32, tag=f"m{hh}", name=f"m{hh}_{k}")
                nc.vector.tensor_mul(mt, dtanh[:, sl], y[:, sl])
                m.append(mt)
            z = ps_loop.tile([H, BL], F32, tag="z", name=f"z_{k}")
            for hh in range(2):
                sl = slice(hh * HALF, (hh + 1) * HALF)
                _mm(nc, z[:, sl], whT_sb, m[hh], start=True, stop=True)
            newg = []
            for hh in range(2):
                sl = slice(hh * HALF, (hh + 1) * HALF)
                gt = loop_sb.tile([H, HALF], F32, tag=f"g{hh}", name=f"g{hh}_{k}")
                nc.vector.tensor_mul(gt, drelu[:, sl], z[:, sl])
                newg.append(gt)
            g = newg

        hdot = acts.tile([H, BL], F32)
        for hh in range(2):
            sl = slice(hh * HALF, (hh + 1) * HALF)
            nc.vector.tensor_mul(hdot[:, sl], dtanh[:, sl], S[:, sl])
        nc.sync.dma_start(out=out, in_=hdot)


def build_module():
    nc = bacc.Bacc(
        "TRN2",
        target_bir_lowering=False,
        debug=False,
        enable_asserts=False,
        num_devices=N_CORES,
    )
    hT = nc.dram_tensor("hT", (H, BL), F32, kind="ExternalInput").ap()
    kcT = nc.dram_tensor("kcT", (4 * C, BL), F32, kind="ExternalInput").ap()
    dkcT = nc.dram_tensor("dkcT", (4 * C, BL), F32, kind="ExternalInput").ap()
    P = nc.dram_tensor("P", (4 * C, H), F32, kind="ExternalInput").ap()
    whT = nc.dram_tensor("whT", (H, H), F32, kind="ExternalInput").ap()
    woutT = nc.dram_tensor("woutT", (H, H), F32, kind="ExternalInput").ap()
    b0 = nc.dram_tensor("b0", (H, 1), F32, kind="ExternalInput").ap()
    b1 = nc.dram_tensor("b1", (H, 1), F32, kind="ExternalInput").ap()
    out = nc.dram_tensor("out", (H, BL), F32, kind="ExternalOutput").ap()

    with tile.TileContext(nc) as tc:
        _body(tc, out, hT, kcT, dkcT, P, whT, woutT, b0, b1)
    nc.compile()
    return nc


_NC_CACHE = None


def _get_module():
    global _NC_CACHE
    if _NC_CACHE is None:
        _NC_CACHE = build_module()
    return _NC_CACHE


def make_in_maps(inputs):
    """Host-side prep: spline interval select + layout transposes + shard."""
    t = np.asarray(inputs["t"], dtype=np.float32)
    h = np.asarray(inputs["h"], dtype=np.float32)
    coeffs = np.asarray(inputs["coeffs"], dtype=np.float32)
    dcoeffs = np.asarray(inputs["dcoeffs"], dtype=np.float32)
    tobs = np.asarray(inputs["tobs"], dtype=np.float32)
    wx = np.asarray(inputs["wx"], dtype=np.float32)
    wh = np.asarray(inputs["wh"], dtype=np.float32)
    wout = np.asarray(inputs["wout"], dtype=np.float32)
    b0 = np.asarray(inputs["b0"], dtype=np.float32)
    b1 = np.asarray(inputs["b1"], dtype=np.float32)

    ts = t[0]
    idx = int(np.clip(np.searchsorted(tobs, ts, side="right") - 1, 0, tobs.shape[0] - 2))
    dt = np.float32(ts) - tobs[idx]

    # P = A(dt) @ wx.T : row (k*32+c) of P is dt^k * wx[:, c]
    dtk = np.float64(dt)
    P_host = np.vstack(
        [(dtk**k) * wx.T.astype(np.float64) for k in range(4)]
    ).astype(np.float32)
    whT = np.ascontiguousarray(wh.T)
    woutT = np.ascontiguousarray(wout.T)
    b0c = np.ascontiguousarray(b0.reshape(H, 1))
    b1c = np.ascontiguousarray(b1.reshape(H, 1))

    co = coeffs[:, idx].reshape(B, 4 * C)
    dco = dcoeffs[:, idx].reshape(B, 4 * C)

    in_maps = []
    for cix in range(N_CORES):
        sl = slice(cix * BL, (cix + 1) * BL)
        in_maps.append(
            {
                "hT": np.ascontiguousarray(h[sl].T),
                "kcT": np.ascontiguousarray(co[sl].T),
                "dkcT": np.ascontiguousarray(dco[sl].T),
                "P": P_host,
                "whT": whT,
                "woutT": woutT,
                "b0": b0c,
                "b1": b1c,
            }
        )
    return in_maps


def run(inputs, trace=False):
    """Run on the 8 NeuronCores. Returns (h_dot [4096,128] f32, exec_time_ns)."""
    in_maps = make_in_maps(inputs)
    nc = _get_module()
    res = bass_utils.run_bass_kernel_spmd(
        nc, in_maps, core_ids=list(range(N_CORES)), trace=trace
    )
    outs = [res.results[cix]["out"] for cix in range(N_CORES)]
    h_dot = np.concatenate([np.asarray(o).T for o in outs], axis=0)
    return np.ascontiguousarray(h_dot, dtype=np.float32), res.exec_time_ns


def kernel(**inputs):
    h_dot, _ = run(inputs, trace=False)
    return h_dot



# revision 2
# speedup vs baseline: 1.1962x; 1.1962x over previous
"""Trainium2 Bass kernel for nn_JaCDEManual_13829794693220.

Computes h_dot for the RNN-cell Jacobian Neumann series:
    x    = cubic_spline(coeffs, tobs, t)           [B, C]
    xdot = cubic_spline(dcoeffs, tobs, t)          [B, C]
    l1   = x @ wx.T + h @ wh.T + b0                [B, H]
    tanh = tanh(relu(l1) @ wout.T + b1)
    d_outer = diag(1-tanh^2) wout diag(sigmoid(l1))   (per batch row)
    h_dot = sum_{k=0..8} (d_outer wh)^k (d_outer wx xdot)

Key algebra: d_outer @ v = dtanh * (wout @ (drelu * v)), so no [B,H,H]
tensor is ever materialized; everything is [128,128] @ [128,256] matmuls
plus elementwise scalings.  S = sum_k wout @ g_k accumulates in PSUM via
duplicate matmuls (cheap on PE); h_dot = dtanh * S at the end.

Matmul operands use float32r (fp32 rounded to 11-bit mantissa, streamed
1 cyc/row on the PE for free dim >= 256 vs 4 cyc/row for fp32).  The BIR
verifier requires every producer of an fp32r-matmul input to emit rounded
values: DMA'd tensors are pre-rounded on the host, on-chip producers
(ACT relu, DVE m/g muls) write float32r-dtype tiles directly.

Sharding: pure data parallel over batch B=4096 -> 8 cores x 512 rows.
Activations live transposed on chip ([H=128 partitions, batch free]); the
host pre-transposes the per-core input slices / weights (layout only) and
folds the degree-4 spline combination matrix A(dt) into P = A @ wx.T so
the spline eval + wx projection is a single matmul per tensor.  Inputs
arrive in 3 coalesced DMAs (weights blob + one data blob per batch half)
instead of 9 small ones.
"""

import os
import sys

import numpy as np

for _p in (
    "/root/.axon_site",
    "/root/.axon_site/_ro/trn_rl_repo",
    "/root/.axon_site/_ro/pypackages",
    "/opt/trn_rl_repo",
):
    if os.path.isdir(_p) and _p not in sys.path:
        sys.path.append(_p)

import concourse.bacc as bacc
import concourse.mybir as mybir
import concourse.tile as tile
from concourse import bass_utils

B, H, C = 4096, 128, 32
N_CORES = 8
BL = B // N_CORES  # 512 batch rows per core
HALF = BL // 2
K_TERMS = 8
F32 = mybir.dt.float32
AF = mybir.ActivationFunctionType

# Matmul-operand dtype mode:
#   fp32r: fp32 with 11-bit mantissa, 1 cyc/row (free dim >= 256). Default.
#   bf16:  1 cyc/row, 8-bit mantissa (fallback if fp32r paths fail).
#   fp32:  exact, 4 cyc/row (2 half-speed passes). Slowest, for debugging.
MM_MODE = os.environ.get("KERNEL_MM_DTYPE", "fp32r")
MM_DT = {
    "fp32": mybir.dt.float32,
    "fp32r": mybir.dt.float32r,
    "bf16": mybir.dt.bfloat16,
}[MM_MODE]
MM_WORDS = 1 if MM_MODE == "bf16" else 1  # free-dim elem count multiplier


def _round_mm(x):
    """Host-side cast of matmul operands to the on-chip operand dtype."""
    x = np.ascontiguousarray(x, dtype=np.float32)
    if MM_MODE == "fp32r":
        u = x.view(np.uint32)
        lsb = (u >> np.uint32(12)) & np.uint32(1)
        u = (u + np.uint32(0x7FF) + lsb) & np.uint32(0xFFFFF000)
        return u.view(np.float32)
    if MM_MODE == "bf16":
        import ml_dtypes

        return x.astype(ml_dtypes.bfloat16)
    return x


def _np_mm_dtype():
    if MM_MODE == "bf16":
        import ml_dtypes

        return ml_dtypes.bfloat16
    return np.float32


# weights blob layout (free-dim offsets into a [128, 386] tensor)
WB_P = 0
WB_WH = 128
WB_WOUT = 256
WB_B0 = 384
WB_B1 = 385
WB_W = 386

# per-half data blob layout (free-dim offsets into a [128, 3*HALF] tensor)
DB_KC = 0
DB_DKC = HALF
DB_H = 2 * HALF
DB_W = 3 * HALF


def _body(tc, out, wblob, dblob0, dblob1):
    from contextlib import ExitStack

    nc = tc.nc
    with ExitStack() as ctx:
        const = ctx.enter_context(tc.tile_pool(name="const", bufs=1))
        data = ctx.enter_context(tc.tile_pool(name="data", bufs=1))
        acts = ctx.enter_context(tc.tile_pool(name="acts", bufs=1))
        loop_sb = ctx.enter_context(tc.tile_pool(name="loop_sb", bufs=2))
        ps_pre = ctx.enter_context(tc.tile_pool(name="ps_pre", bufs=1, space="PSUM"))
        ps_y = ctx.enter_context(tc.tile_pool(name="ps_y", bufs=2, space="PSUM"))
        ps_z = ctx.enter_context(tc.tile_pool(name="ps_z", bufs=2, space="PSUM"))
        ps_s = ctx.enter_context(tc.tile_pool(name="ps_s", bufs=1, space="PSUM"))

        wb = const.tile([128, WB_W], MM_DT)
        nc.sync.dma_start(out=wb, in_=wblob)
        db = [data.tile([128, DB_W], MM_DT, name=f"db{h}") for h in range(2)]
        nc.sync.dma_start(out=db[0], in_=dblob0)
        nc.sync.dma_start(out=db[1], in_=dblob1)

        P_sb = wb[:, WB_P : WB_P + 128]
        whT_sb = wb[:, WB_WH : WB_WH + 128]
        woutT_sb = wb[:, WB_WOUT : WB_WOUT + 128]
        b0_sb = wb[:, WB_B0 : WB_B0 + 1]
        b1_sb = wb[:, WB_B1 : WB_B1 + 1]
        if MM_MODE == "fp32r":
            b0_sb = b0_sb.bitcast(F32)
            b1_sb = b1_sb.bitcast(F32)

        # --- pre-stage (per batch half): l1, u, relu, sigmoid, a2, tanh,
        # dtanh, g0.  l1.T = (wx A^T) kcT + wh hT ; u.T = (wx A^T) dkcT
        l1 = ps_pre.tile([H, BL], F32, tag="l1")
        u = ps_pre.tile([H, BL], F32, tag="u")
        a2 = ps_pre.tile([H, BL], F32, tag="a2")
        relu = acts.tile([H, BL], MM_DT)
        drelu = acts.tile([H, BL], F32)
        tanh_sb = acts.tile([H, BL], F32)
        dtanh = acts.tile([H, BL], F32)

        for h in range(2):
            sl = slice(h * HALF, (h + 1) * HALF)
            kc = db[h][:, DB_KC : DB_KC + HALF]
            dkc = db[h][:, DB_DKC : DB_DKC + HALF]
            hT = db[h][:, DB_H : DB_H + HALF]
            nc.tensor.matmul(l1[:, sl], P_sb, kc, start=True, stop=False)
            nc.tensor.matmul(l1[:, sl], whT_sb, hT, start=False, stop=True)
            nc.tensor.matmul(u[:, sl], P_sb, dkc, start=True, stop=True)
            nc.scalar.activation(relu[:, sl], l1[:, sl], AF.Relu, bias=b0_sb)
            nc.scalar.activation(drelu[:, sl], l1[:, sl], AF.Sigmoid, bias=b0_sb)
            nc.tensor.matmul(a2[:, sl], woutT_sb, relu[:, sl], start=True, stop=True)
            nc.scalar.activation(tanh_sb[:, sl], a2[:, sl], AF.Tanh, bias=b1_sb)
            # dtanh = 1 - tanh^2: square on ACT, then (x * -1 + 1) on DVE
            nc.scalar.activation(dtanh[:, sl], tanh_sb[:, sl], AF.Square)
            nc.vector.tensor_scalar(
                out=dtanh[:, sl],
                in0=dtanh[:, sl],
                scalar1=-1.0,
                scalar2=1.0,
                op0=mybir.AluOpType.mult,
                op1=mybir.AluOpType.add,
            )

        g = []
        for h in range(2):
            sl = slice(h * HALF, (h + 1) * HALF)
            gt = loop_sb.tile([H, HALF], MM_DT, tag=f"g{h}", name=f"g{h}_init")
            nc.vector.tensor_mul(gt, drelu[:, sl], u[:, sl])
            g.append(gt)

        # --- Neumann loop.  S accumulates sum_k wout @ g_k in PSUM via
        # duplicate matmuls; h_dot = dtanh * S at the end.
        S = ps_s.tile([H, BL], F32, tag="S")
        for k in range(K_TERMS + 1):
            last = k == K_TERMS
            y = None
            if not last:
                y = ps_y.tile([H, BL], F32, tag="y", name=f"y_{k}")
                for h in range(2):
                    sl = slice(h * HALF, (h + 1) * HALF)
                    nc.tensor.matmul(y[:, sl], woutT_sb, g[h], start=True, stop=True)
            for h in range(2):
                sl = slice(h * HALF, (h + 1) * HALF)
                # start only once: start=True marks the whole 2KB PSUM zero
                # region pending-zero, so a second start on this bank would
                # wipe the other half's partial sum.
                nc.tensor.matmul(
                    S[:, sl],
                    woutT_sb,
                    g[h],
                    start=(k == 0 and h == 0),
                    stop=(last and h == 1),
                )
            if last:
                break
            m = []
            for h in range(2):
                sl = slice(h * HALF, (h + 1) * HALF)
                mt = loop_sb.tile([H, HALF], MM_DT, tag=f"m{h}", name=f"m{h}_{k}")
                nc.vector.tensor_mul(mt, dtanh[:, sl], y[:, sl])
                m.append(mt)
            z = ps_z.tile([H, BL], F32, tag="z", name=f"z_{k}")
            for h in range(2):
                sl = slice(h * HALF, (h + 1) * HALF)
                nc.tensor.matmul(z[:, sl], whT_sb, m[h], start=True, stop=True)
            newg = []
            for h in range(2):
                sl = slice(h * HALF, (h + 1) * HALF)
                gt = loop_sb.tile([H, HALF], MM_DT, tag=f"g{h}", name=f"g{h}_{k}")
                nc.vector.tensor_mul(gt, drelu[:, sl], z[:, sl])
                newg.append(gt)
            g = newg

        hdot = acts.tile([H, BL], F32)
        for h in range(2):
            sl = slice(h * HALF, (h + 1) * HALF)
            nc.vector.tensor_mul(hdot[:, sl], dtanh[:, sl], S[:, sl])
        nc.sync.dma_start(out=out, in_=hdot)


def build_module():
    nc = bacc.Bacc(
        "TRN2",
        target_bir_lowering=False,
        debug=False,
        enable_asserts=False,
        num_devices=N_CORES,
    )
    wblob = nc.dram_tensor("wblob", (128, WB_W), MM_DT, kind="ExternalInput").ap()
    dblob0 = nc.dram_tensor("dblob0", (128, DB_W), MM_DT, kind="ExternalInput").ap()
    dblob1 = nc.dram_tensor("dblob1", (128, DB_W), MM_DT, kind="ExternalInput").ap()
    out = nc.dram_tensor("out", (H, BL), F32, kind="ExternalOutput").ap()

    with tile.TileContext(nc) as tc:
        _body(tc, out, wblob, dblob0, dblob1)
    nc.compile()
    return nc


_NC_CACHE = None


def _get_module():
    global _NC_CACHE
    if _NC_CACHE is None:
        _NC_CACHE = build_module()
    return _NC_CACHE


def make_in_maps(inputs):
    """Host-side prep: spline interval select + layout transposes + shard."""
    t = np.asarray(inputs["t"], dtype=np.float32)
    h = np.asarray(inputs["h"], dtype=np.float32)
    coeffs = np.asarray(inputs["coeffs"], dtype=np.float32)
    dcoeffs = np.asarray(inputs["dcoeffs"], dtype=np.float32)
    tobs = np.asarray(inputs["tobs"], dtype=np.float32)
    wx = np.asarray(inputs["wx"], dtype=np.float32)
    wh = np.asarray(inputs["wh"], dtype=np.float32)
    wout = np.asarray(inputs["wout"], dtype=np.float32)
    b0 = np.asarray(inputs["b0"], dtype=np.float32)
    b1 = np.asarray(inputs["b1"], dtype=np.float32)

    ts = t[0]
    idx = int(np.clip(np.searchsorted(tobs, ts, side="right") - 1, 0, tobs.shape[0] - 2))
    dt = np.float32(ts) - tobs[idx]

    # P = A(dt) @ wx.T : row (k*32+c) of P is dt^k * wx[:, c]
    dtk = np.float64(dt)
    P_host = np.vstack(
        [(dtk**k) * wx.T.astype(np.float64) for k in range(4)]
    ).astype(np.float32)

    npdt = _np_mm_dtype()
    wblob = np.zeros((128, WB_W), dtype=npdt)
    wblob[:, WB_P : WB_P + 128] = _round_mm(P_host)
    wblob[:, WB_WH : WB_WH + 128] = _round_mm(wh.T)
    wblob[:, WB_WOUT : WB_WOUT + 128] = _round_mm(wout.T)
    if MM_MODE == "fp32r":
        # biases ride in the f32r blob as raw fp32 bits (bitcast on chip)
        wblob[:, WB_B0] = b0.view(npdt) if npdt is np.float32 else b0
        wblob[:, WB_B1] = b1.view(npdt) if npdt is np.float32 else b1
    else:
        wblob[:, WB_B0] = b0.astype(npdt)
        wblob[:, WB_B1] = b1.astype(npdt)

    co = _round_mm(coeffs[:, idx].reshape(B, 4 * C).T)  # [128, B]
    dco = _round_mm(dcoeffs[:, idx].reshape(B, 4 * C).T)
    hT = _round_mm(h.T)  # [128, B]

    in_maps = []
    for cix in range(N_CORES):
        base = cix * BL
        m = {"wblob": wblob}
        for hh in range(2):
            sl = slice(base + hh * HALF, base + (hh + 1) * HALF)
            blob = np.empty((128, DB_W), dtype=npdt)
            blob[:, DB_KC : DB_KC + HALF] = co[:, sl]
            blob[:, DB_DKC : DB_DKC + HALF] = dco[:, sl]
            blob[:, DB_H : DB_H + HALF] = hT[:, sl]
            m[f"dblob{hh}"] = blob
        in_maps.append(m)
    return in_maps


def run(inputs, trace=False):
    """Run on the 8 NeuronCores. Returns (h_dot [4096,128] f32, exec_time_ns)."""
    in_maps = make_in_maps(inputs)
    nc = _get_module()
    res = bass_utils.run_bass_kernel_spmd(
        nc, in_maps, core_ids=list(range(N_CORES)), trace=trace
    )
    outs = [res.results[cix]["out"] for cix in range(N_CORES)]
    h_dot = np.concatenate([np.asarray(o).T for o in outs], axis=0)
    return np.ascontiguousarray(h_dot, dtype=np.float32), res.exec_time_ns


def kernel(**inputs):
    h_dot, _ = run(inputs, trace=False)
    return h_dot
